# revision 4
# baseline (speedup 1.0000x reference)
"""Trainium2 Bass kernel for the NiN-Conv2D problem.

Network: per-pixel MLP over 7x7x3 patches, independent per filter f:
  h0 = relu(P @ W0[:,:,f] + b0)   (147 -> 32)
  h1 = relu(h0 @ W1[:,:,f] + b1)  (32 -> 16)
  out = relu(h1 @ W2[:,:,f] + b2) (16 -> 1)
for B=32, H=W=32, F=128.

Strategy: data-parallel over batch across 8 NeuronCores (4 images each).
On each core everything runs in a "feature-major" orientation: activations
live as (d*f on partitions, pixels on free dim), weights are the stationary
matmul operand, so no transposes are needed between layers.

  L0: per group of 4 filters, lhsT = W0 chunk (K=128 / K=19 accumulated),
      rhs = im2col-transposed patches (K, pix) -> PSUM (128=4f*32, pix)
  L1: per pair of groups, block-diag W1 (128, 64), two matmuls fill the
      two partition halves of one PSUM tile -> (128=8f*16, pix)
  L2: per 32-filter block, 4 accumulating block-diag matmuls -> (f, pix)

Bias+ReLU+cast(PSUM->SBUF) fused into one ACT/DVE op, split across both
engines to balance load. Matmul operands are bf16 (fp32 PSUM accumulate).

Host side: im2col transpose + weight packing (pure layout, no FLOPs).
"""
import numpy as np
import ml_dtypes

import concourse.bass as bass
import concourse.mybir as mybir
from concourse import bacc, tile
from concourse import bass_utils
from concourse.bass import ts

KH, KW = 7, 7
B, H, W, C, F = 32, 32, 32, 3, 128
K, D0, D1 = 147, 32, 16
NCORES = 8
BPC = B // NCORES            # 4 images per core
NPIX = BPC * H * W           # 4096 pixels per core
PTILE = 512
NT = NPIX // PTILE           # 8 pixel tiles

BF16 = mybir.dt.bfloat16
F32 = mybir.dt.float32
NPBF16 = ml_dtypes.bfloat16

# fraction of relu/bias ops routed to ScalarE (rest go to VectorE)
ACT_SPLIT = {"h0": 2, "h1": 2}   # 1 of every ACT_SPLIT[..] ops on ScalarE


# ----------------------------------------------------------------------------
# host-side packing (layout only)
# ----------------------------------------------------------------------------

def _pack_weights(w0, b0, w1, b1, w2, b2):
    """Shared (core-independent) weight/bias packing. Returns dict of np arrays."""
    w0 = np.asarray(w0, np.float32)
    w1 = np.asarray(w1, np.float32)
    w2 = np.asarray(w2, np.float32)
    b0 = np.asarray(b0, np.float32)
    b1 = np.asarray(b1, np.float32)
    b2 = np.asarray(b2, np.float32)

    w0a = np.empty((128, 32, 128), np.float32)   # [k, group, m=fl*32+d]
    w0b = np.empty((19, 32, 128), np.float32)
    b0s = np.empty((128, 32), np.float32)
    for g in range(32):
        m = w0[:, :, 4 * g:4 * g + 4].transpose(0, 2, 1).reshape(K, 128)
        w0a[:, g, :] = m[:128]
        w0b[:, g, :] = m[128:]
        b0s[:, g] = b0[:, 4 * g:4 * g + 4].T.reshape(128)

    w1bd = np.zeros((128, 32, 64), np.float32)   # [k=fl*32+d0, g, m=fl*16+d1]
    b1s = np.empty((128, 16), np.float32)
    for g in range(32):
        for fl in range(4):
            f = 4 * g + fl
            w1bd[fl * 32:(fl + 1) * 32, g, fl * 16:(fl + 1) * 16] = w1[:, :, f]
    for p in range(16):
        for half in range(2):
            g = 2 * p + half
            b1s[half * 64:(half + 1) * 64, p] = b1[:, 4 * g:4 * g + 4].T.reshape(64)

    w2bd = np.zeros((128, 16, 32), np.float32)   # [k=half*64+fl*16+d1, pair, col]
    for p in range(16):
        for half in range(2):
            for fl in range(4):
                f = 8 * p + half * 4 + fl
                col = f - 32 * (p // 4)
                w2bd[half * 64 + fl * 16:half * 64 + (fl + 1) * 16, p, col] = w2[:, 0, f]
    b2s = b2.reshape(128, 1).astype(np.float32)

    return {
        "w0a": w0a.reshape(128, 4096).astype(NPBF16),
        "w0b": w0b.reshape(19, 4096).astype(NPBF16),
        "w1bd": w1bd.reshape(128, 2048).astype(NPBF16),
        "w2bd": w2bd.reshape(128, 512).astype(NPBF16),
        "b0s": b0s, "b1s": b1s, "b2s": b2s,
    }


def _im2col_T(x_core):
    """x_core (4,32,32,3) fp32 -> PT (147, 4096) with k=(kh*7+kw)*3+c."""
    xp = np.pad(np.asarray(x_core, np.float32), ((0, 0), (3, 3), (3, 3), (0, 0)))
    PT = np.empty((K, NPIX), np.float32)
    for kh in range(KH):
        for kw in range(KW):
            blk = xp[:, kh:kh + H, kw:kw + W, :]
            t = kh * 7 + kw
            PT[t * 3:t * 3 + 3] = blk.transpose(3, 0, 1, 2).reshape(3, NPIX)
    return PT


# ----------------------------------------------------------------------------
# device kernel
# ----------------------------------------------------------------------------

def _body(tc):
    nc = tc.nc
    Relu = mybir.ActivationFunctionType.Relu
    Add, Max = mybir.AluOpType.add, mybir.AluOpType.max

    pt1 = nc.dram_tensor("pt1", [128, NPIX], BF16, kind="ExternalInput").ap()
    pt2 = nc.dram_tensor("pt2", [19, NPIX], BF16, kind="ExternalInput").ap()
    w0a = nc.dram_tensor("w0a", [128, 4096], BF16, kind="ExternalInput").ap()
    w0b = nc.dram_tensor("w0b", [19, 4096], BF16, kind="ExternalInput").ap()
    w1bd = nc.dram_tensor("w1bd", [128, 2048], BF16, kind="ExternalInput").ap()
    w2bd = nc.dram_tensor("w2bd", [128, 512], BF16, kind="ExternalInput").ap()
    b0d = nc.dram_tensor("b0s", [128, 32], F32, kind="ExternalInput").ap()
    b1d = nc.dram_tensor("b1s", [128, 16], F32, kind="ExternalInput").ap()
    b2d = nc.dram_tensor("b2s", [128, 1], F32, kind="ExternalInput").ap()
    out = nc.dram_tensor("out", [128, NPIX], F32, kind="ExternalOutput").ap()

    with (
        tc.tile_pool(name="consts", bufs=1) as cpool,
        tc.tile_pool(name="h0", bufs=40) as h0pool,
        tc.tile_pool(name="h1", bufs=20) as h1pool,
        tc.tile_pool(name="outs", bufs=3) as opool,
        tc.tile_pool(name="l0p", bufs=3, space="PSUM") as l0pool,
        tc.tile_pool(name="l1p", bufs=2, space="PSUM") as l1pool,
        tc.tile_pool(name="l2p", bufs=2, space="PSUM") as l2pool,
    ):
        def load(ap, shape, dt, tag):
            t = cpool.tile(shape, dt, tag=tag)
            nc.sync.dma_start(t[:], ap)
            return t

        pt1s = load(pt1, [128, NPIX], BF16, "pt1")
        pt2s = load(pt2, [19, NPIX], BF16, "pt2")
        w0as = load(w0a, [128, 4096], BF16, "w0a")
        w0bs = load(w0b, [19, 4096], BF16, "w0b")
        w1s = load(w1bd, [128, 2048], BF16, "w1")
        w2s = load(w2bd, [128, 512], BF16, "w2")
        b0s = load(b0d, [128, 32], F32, "b0")
        b1s = load(b1d, [128, 16], F32, "b1")
        b2s = load(b2d, [128, 1], F32, "b2")

        def relu_copy(dst, src, bias, which, idx):
            if idx % ACT_SPLIT[which] == 0:
                nc.scalar.activation(dst, src, Relu, bias=bias)
            else:
                nc.vector.tensor_scalar(dst, src, bias, 0.0, Add, Max)

        for t in range(NT):
            pix = ts(t, PTILE)
            # ---- layer 0: 32 groups of 4 filters
            h0 = []
            for g in range(32):
                ps = l0pool.tile([128, PTILE], F32, tag="l0")
                nc.tensor.matmul(ps[:], w0as[:, ts(g, 128)], pt1s[:, pix],
                                 start=True, stop=False)
                nc.tensor.matmul(ps[:], w0bs[:, ts(g, 128)], pt2s[:, pix],
                                 start=False, stop=True)
                h = h0pool.tile([128, PTILE], BF16, tag="h0")
                relu_copy(h[:], ps[:], b0s[:, g:g + 1], "h0", g)
                h0.append(h)
            # ---- layer 1: 16 pairs of groups -> (128 = 8f*16, pix)
            h1 = []
            for p in range(16):
                ps = l1pool.tile([128, PTILE], F32, tag="l1")
                nc.tensor.matmul(ps[0:64, :], w1s[:, ts(2 * p, 64)],
                                 h0[2 * p][:], start=True, stop=True)
                nc.tensor.matmul(ps[64:128, :], w1s[:, ts(2 * p + 1, 64)],
                                 h0[2 * p + 1][:], start=True, stop=True)
                h = h1pool.tile([128, PTILE], BF16, tag="h1")
                relu_copy(h[:], ps[:], b1s[:, p:p + 1], "h1", p)
                h1.append(h)
            # ---- layer 2: 4 blocks of 32 filters, 4 accumulating chunks each
            ps2 = l2pool.tile([128, PTILE], F32, tag="l2")
            for jj in range(4):
                for q in range(4):
                    p = 4 * jj + q
                    nc.tensor.matmul(ps2[32 * jj:32 * jj + 32, :],
                                     w2s[:, ts(p, 32)], h1[p][:],
                                     start=(q == 0), stop=(q == 3),
                                     tile_position=(0, 32 * jj))
            ot = opool.tile([128, PTILE], F32, tag="o")
            nc.scalar.activation(ot[:], ps2[:], Relu, bias=b2s[:, 0:1])
            nc.sync.dma_start(out[:, pix], ot[:])


_COMPILED = None


def _get_compiled():
    global _COMPILED
    if _COMPILED is None:
        import time as _time
        t0 = _time.time()
        nc = bacc.Bacc("TRN2", target_bir_lowering=False, debug=False,
                       num_devices=NCORES)
        with tile.TileContext(nc) as tc:
            _body(tc)
        t1 = _time.time()
        nc.compile()
        t2 = _time.time()
        print(f"[kernel] tile build+schedule {t1 - t0:.1f}s, bacc compile {t2 - t1:.1f}s",
              flush=True)
        _COMPILED = nc
    return _COMPILED


# ----------------------------------------------------------------------------
# public entry point
# ----------------------------------------------------------------------------

def kernel(x, w0, b0, w1, b1, w2, b2, _trace=False):
    x = np.asarray(x, np.float32)
    shared = _pack_weights(w0, b0, w1, b1, w2, b2)

    in_maps = []
    for k in range(NCORES):
        PT = _im2col_T(x[BPC * k:BPC * (k + 1)])
        m = dict(shared)
        m["pt1"] = PT[:128].astype(NPBF16)
        m["pt2"] = PT[128:].astype(NPBF16)
        in_maps.append(m)

    import time as _time
    nc = _get_compiled()
    t0 = _time.time()
    res = bass_utils.run_bass_kernel_spmd(
        nc, in_maps, core_ids=list(range(NCORES)), trace=_trace)
    print(f"[kernel] run_bass_kernel_spmd {_time.time() - t0:.1f}s", flush=True)

    outs = []
    for k in range(NCORES):
        oc = res.results[k]["out"]                     # (128, 4096) fp32
        outs.append(oc.reshape(F, BPC, H, W).transpose(1, 2, 3, 0))
    full = np.concatenate(outs, axis=0).astype(np.float32)
    if _trace:
        return full, res
    return full


# revision 14
# speedup vs baseline: 1.4162x; 1.4162x over previous
"""Trainium2 Bass kernel for the NiN-Conv2D problem.

Network: per-pixel MLP over 7x7x3 patches, independent per filter f:
  h0 = relu(P @ W0[:,:,f] + b0)   (147 -> 32)
  h1 = relu(h0 @ W1[:,:,f] + b1)  (32 -> 16)
  out = relu(h1 @ W2[:,:,f] + b2) (16 -> 1)
for B=32, H=W=32, F=128.

Strategy: data-parallel over batch across 8 NeuronCores (4 images each).
On each core everything runs in a "feature-major" orientation: activations
live as (d*f on partitions, pixels on free dim), weights are the stationary
matmul operand, so no transposes are needed between layers.

  L0: per group of 4 filters, lhsT = W0 chunk (K=128 / K=19 accumulated),
      rhs = im2col-transposed patches (K, pix) -> PSUM (128=4f*32, pix)
  L1: per pair of groups, block-diag W1 (128, 64), two matmuls fill the
      two partition halves of one PSUM tile -> (128=8f*16, pix)
  L2: per 32-filter block, 4 accumulating block-diag matmuls -> (f, pix)

Bias+ReLU+cast(PSUM->SBUF) fused into one ACT/DVE op, split across both
engines to balance load. Matmul operands are bf16 (fp32 PSUM accumulate).

Host side: im2col transpose + weight packing (pure layout, no FLOPs).
"""
import numpy as np
import ml_dtypes

import concourse.bass as bass
import concourse.mybir as mybir
from concourse import bacc, tile
from concourse import bass_utils
from concourse.bass import ts

KH, KW = 7, 7
B, H, W, C, F = 32, 32, 32, 3, 128
K, D0, D1 = 147, 32, 16
NCORES = 8
BPC = B // NCORES            # 4 images per core
NPIX = BPC * H * W           # 4096 pixels per core
PTILE = 512
NT = NPIX // PTILE           # 8 pixel tiles

BF16 = mybir.dt.bfloat16
F32 = mybir.dt.float32
NPBF16 = ml_dtypes.bfloat16

# fraction of relu/bias ops routed to ScalarE (rest go to VectorE)
ACT_SPLIT = {"h0": 2, "h1": 2}   # 1 of every ACT_SPLIT[..] ops on ScalarE


# ----------------------------------------------------------------------------
# host-side packing (layout only)
# ----------------------------------------------------------------------------

def _pack_weights(w0, b0, w1, b1, w2, b2):
    """Shared (core-independent) weight/bias packing. Returns dict of np arrays."""
    w0 = np.asarray(w0, np.float32)
    w1 = np.asarray(w1, np.float32)
    w2 = np.asarray(w2, np.float32)
    b0 = np.asarray(b0, np.float32)
    b1 = np.asarray(b1, np.float32)
    b2 = np.asarray(b2, np.float32)

    w0a = np.empty((128, 32, 128), np.float32)   # [k, group, m=fl*32+d]
    # chunk2 (K rows 128..146 + bias row) packed for 4-way row-tiled
    # concurrency: group g lives at partitions 32*(g%4)+k, cols g*128+m.
    # Row 32*(g%4)+19 carries b0 (the patch tile has ones there), so the
    # PSUM result already includes the bias and the relu op needs none.
    w0b = np.zeros((128, 32, 128), np.float32)
    for g in range(32):
        m = w0[:, :, 4 * g:4 * g + 4].transpose(0, 2, 1).reshape(K, 128)
        w0a[:, g, :] = m[:128]
        r = g % 4
        w0b[32 * r:32 * r + 19, g, :] = m[128:]
        w0b[32 * r + 19, g, :] = b0[:, 4 * g:4 * g + 4].T.reshape(128)

    w1bd = np.zeros((128, 32, 64), np.float32)   # [k=fl*32+d0, g, m=fl*16+d1]
    b1s = np.empty((128, 16), np.float32)
    for g in range(32):
        for fl in range(4):
            f = 4 * g + fl
            w1bd[fl * 32:(fl + 1) * 32, g, fl * 16:(fl + 1) * 16] = w1[:, :, f]
    for p in range(16):
        for half in range(2):
            g = 2 * p + half
            b1s[half * 64:(half + 1) * 64, p] = b1[:, 4 * g:4 * g + 4].T.reshape(64)

    w2bd = np.zeros((128, 16, 32), np.float32)   # [k=half*64+fl*16+d1, pair, col]
    for p in range(16):
        for half in range(2):
            for fl in range(4):
                f = 8 * p + half * 4 + fl
                col = f - 32 * (p // 4)
                w2bd[half * 64 + fl * 16:half * 64 + (fl + 1) * 16, p, col] = w2[:, 0, f]
    b2s = b2.reshape(128, 1).astype(np.float32)

    return {
        "w0a": w0a.reshape(128, 4096).astype(NPBF16),
        "w0b": w0b.reshape(128, 4096).astype(NPBF16),
        "w1bd": w1bd.reshape(128, 2048).astype(NPBF16),
        "w2bd": w2bd.reshape(128, 512).astype(NPBF16),
        "b1s": b1s, "b2s": b2s,
    }


def _im2col_T(x_core):
    """x_core (4,32,32,3) fp32 -> PT (147, 4096) with k=(kh*7+kw)*3+c."""
    xp = np.pad(np.asarray(x_core, np.float32), ((0, 0), (3, 3), (3, 3), (0, 0)))
    PT = np.empty((K, NPIX), np.float32)
    for kh in range(KH):
        for kw in range(KW):
            blk = xp[:, kh:kh + H, kw:kw + W, :]
            t = kh * 7 + kw
            PT[t * 3:t * 3 + 3] = blk.transpose(3, 0, 1, 2).reshape(3, NPIX)
    return PT


# ----------------------------------------------------------------------------
# device kernel
# ----------------------------------------------------------------------------

def _body(tc):
    nc = tc.nc
    Relu = mybir.ActivationFunctionType.Relu
    Add, Max = mybir.AluOpType.add, mybir.AluOpType.max

    pt1 = nc.dram_tensor("pt1", [128, NPIX], BF16, kind="ExternalInput").ap()
    pt2 = nc.dram_tensor("pt2", [128, NPIX], BF16, kind="ExternalInput").ap()
    w0a = nc.dram_tensor("w0a", [128, 4096], BF16, kind="ExternalInput").ap()
    w0b = nc.dram_tensor("w0b", [128, 4096], BF16, kind="ExternalInput").ap()
    w1bd = nc.dram_tensor("w1bd", [128, 2048], BF16, kind="ExternalInput").ap()
    w2bd = nc.dram_tensor("w2bd", [128, 512], BF16, kind="ExternalInput").ap()
    b1d = nc.dram_tensor("b1s", [128, 16], F32, kind="ExternalInput").ap()
    b2d = nc.dram_tensor("b2s", [128, 1], F32, kind="ExternalInput").ap()
    out = nc.dram_tensor("out", [128, NPIX], F32, kind="ExternalOutput").ap()

    with (
        tc.tile_pool(name="consts", bufs=1) as cpool,
        tc.tile_pool(name="h0", bufs=20) as h0pool,
        tc.tile_pool(name="h1", bufs=20) as h1pool,
        tc.tile_pool(name="outs", bufs=3) as opool,
        tc.tile_pool(name="l0p", bufs=3, space="PSUM") as l0pool,
        tc.tile_pool(name="l12p", bufs=2, space="PSUM") as l12pool,
    ):
        def load(ap, shape, dt, tag):
            t = cpool.tile(shape, dt, tag=tag)
            nc.sync.dma_start(t[:], ap)
            return t

        w0as = load(w0a, [128, 4096], BF16, "w0a")
        w1s = load(w1bd, [128, 2048], BF16, "w1")
        w2s = load(w2bd, [128, 512], BF16, "w2")
        b1s = load(b1d, [128, 16], F32, "b1")
        b2s = load(b2d, [128, 1], F32, "b2")
        w0bs = load(w0b, [128, 4096], BF16, "w0b")
        pt2s = load(pt2, [128, NPIX], BF16, "pt2")
        # patch tile per pixel-tile so the first matmuls start early
        pt1s = []
        for t in range(NT):
            p = cpool.tile([128, PTILE], BF16, tag=f"pt1_{t}")
            nc.sync.dma_start(p[:], pt1[:, ts(t, PTILE)])
            pt1s.append(p)

        def relu(dst, src, bias, idx):
            # alternate whole tiles between ScalarE and VectorE
            if idx % 2 == 0:
                nc.scalar.activation(dst, src, Relu, bias=bias)
            else:
                nc.vector.tensor_scalar(dst, src, bias, 0.0, Add, Max)

        for t in range(NT):
            pix = ts(t, PTILE)
            # ---- layer 0: 8 quads of 4 filter-groups; two (128,1024) PSUM
            # tiles per quad (2 groups each, one per column half); chunk2
            # (K rows 128..146 + bias row) runs 4-way concurrent via
            # row-group tiling. Bias rides in the matmul, so one wide
            # bias-free relu op covers a whole tile.
            h0 = []       # 16 tiles (128,1024): groups (2j, 2j+1)
            for q in range(8):
                psA = l0pool.tile([128, 2 * PTILE], F32, tag="l0")
                psB = l0pool.tile([128, 2 * PTILE], F32, tag="l0")
                for r in range(4):
                    g = 4 * q + r
                    ps = psA if r < 2 else psB
                    dst = ps[:, ts(r % 2, PTILE)]
                    nc.tensor.matmul(dst, w0as[:, ts(g, 128)], pt1s[t][:],
                                     start=True, stop=False)
                for r in range(4):
                    g = 4 * q + r
                    ps = psA if r < 2 else psB
                    dst = ps[:, ts(r % 2, PTILE)]
                    nc.tensor.matmul(dst, w0bs[32 * r:32 * r + 20, ts(g, 128)],
                                     pt2s[32 * r:32 * r + 20, pix],
                                     start=False, stop=True,
                                     tile_position=(32 * r, 0))
                for j, ps in ((2 * q, psA), (2 * q + 1, psB)):
                    h = h0pool.tile([128, 2 * PTILE], BF16, tag="h0")
                    if j % 2 == 0:
                        nc.scalar.activation(h[:], ps[:], Relu)
                    else:
                        nc.vector.tensor_scalar_max(h[:], ps[:], 0.0)
                    h0.append(h)
            # ---- layer 1: 16 pairs of groups -> (128 = 8f*16, pix)
            h1 = []
            for p in range(16):
                ps = l12pool.tile([128, PTILE], F32, tag="l12")
                nc.tensor.matmul(ps[0:64, :], w1s[:, ts(2 * p, 64)],
                                 h0[p][:, 0:PTILE], start=True, stop=True)
                nc.tensor.matmul(ps[64:128, :], w1s[:, ts(2 * p + 1, 64)],
                                 h0[p][:, PTILE:], start=True, stop=True)
                h = h1pool.tile([128, PTILE], BF16, tag="h1")
                relu(h[:], ps[:], b1s[:, p:p + 1], p)
                h1.append(h)
            # ---- layer 2: 4 blocks of 32 filters; q-major order so the 4
            # blocks' matmuls hit disjoint PE column groups concurrently
            ps2 = l12pool.tile([128, PTILE], F32, tag="l12")
            for q in range(4):
                for jj in range(4):
                    p = 4 * jj + q
                    nc.tensor.matmul(ps2[32 * jj:32 * jj + 32, :],
                                     w2s[:, ts(p, 32)], h1[p][:],
                                     start=(q == 0), stop=(q == 3),
                                     tile_position=(0, 32 * jj))
            ot = opool.tile([128, PTILE], F32, tag="o")
            nc.scalar.activation(ot[:], ps2[:], Relu, bias=b2s[:, 0:1])
            nc.sync.dma_start(out[:, pix], ot[:])


_COMPILED = None


def _get_compiled():
    global _COMPILED
    if _COMPILED is None:
        import time as _time
        t0 = _time.time()
        nc = bacc.Bacc("TRN2", target_bir_lowering=False, debug=False,
                       num_devices=NCORES)
        with tile.TileContext(nc) as tc:
            _body(tc)
        t1 = _time.time()
        nc.compile()
        t2 = _time.time()
        print(f"[kernel] tile build+schedule {t1 - t0:.1f}s, bacc compile {t2 - t1:.1f}s",
              flush=True)
        _COMPILED = nc
    return _COMPILED


# ----------------------------------------------------------------------------
# public entry point
# ----------------------------------------------------------------------------

def kernel(x, w0, b0, w1, b1, w2, b2, _trace=False):
    x = np.asarray(x, np.float32)
    shared = _pack_weights(w0, b0, w1, b1, w2, b2)

    in_maps = []
    for k in range(NCORES):
        PT = _im2col_T(x[BPC * k:BPC * (k + 1)])
        m = dict(shared)
        m["pt1"] = PT[:128].astype(NPBF16)
        # chunk2 rows replicated at partitions 32r (4-way row tiling),
        # with a ones row at 32r+19 that carries b0 through the matmul
        pt2 = np.zeros((128, NPIX), np.float32)
        for r in range(4):
            pt2[32 * r:32 * r + 19] = PT[128:]
            pt2[32 * r + 19] = 1.0
        m["pt2"] = pt2.astype(NPBF16)
        in_maps.append(m)

    import time as _time
    nc = _get_compiled()
    t0 = _time.time()
    res = bass_utils.run_bass_kernel_spmd(
        nc, in_maps, core_ids=list(range(NCORES)), trace=_trace)
    print(f"[kernel] run_bass_kernel_spmd {_time.time() - t0:.1f}s", flush=True)

    outs = []
    for k in range(NCORES):
        oc = res.results[k]["out"]                     # (128, 4096) fp32
        outs.append(oc.reshape(F, BPC, H, W).transpose(1, 2, 3, 0))
    full = np.concatenate(outs, axis=0).astype(np.float32)
    if _trace:
        return full, res
    return full


# revision 17
# speedup vs baseline: 1.4759x; 1.0421x over previous
"""Trainium2 Bass kernel for the NiN-Conv2D problem.

Network: per-pixel MLP over 7x7x3 patches, independent per filter f:
  h0 = relu(P @ W0[:,:,f] + b0)   (147 -> 32)
  h1 = relu(h0 @ W1[:,:,f] + b1)  (32 -> 16)
  out = relu(h1 @ W2[:,:,f] + b2) (16 -> 1)
for B=32, H=W=32, F=128.

Strategy: data-parallel over batch across 8 NeuronCores (4 images each).
On each core everything runs in a "feature-major" orientation: activations
live as (d*f on partitions, pixels on free dim), weights are the stationary
matmul operand, so no transposes are needed between layers.

  L0: per group of 4 filters, lhsT = W0 chunk (K=128 / K=19 accumulated),
      rhs = im2col-transposed patches (K, pix) -> PSUM (128=4f*32, pix)
  L1: per pair of groups, block-diag W1 (128, 64), two matmuls fill the
      two partition halves of one PSUM tile -> (128=8f*16, pix)
  L2: per 32-filter block, 4 accumulating block-diag matmuls -> (f, pix)

Bias+ReLU+cast(PSUM->SBUF) fused into one ACT/DVE op, split across both
engines to balance load. Matmul operands are bf16 (fp32 PSUM accumulate).

Host side: im2col transpose + weight packing (pure layout, no FLOPs).
"""
import numpy as np
import ml_dtypes

import concourse.bass as bass
import concourse.mybir as mybir
from concourse import bacc, tile
from concourse import bass_utils
from concourse.bass import ts

KH, KW = 7, 7
B, H, W, C, F = 32, 32, 32, 3, 128
K, D0, D1 = 147, 32, 16
NCORES = 8
BPC = B // NCORES            # 4 images per core
NPIX = BPC * H * W           # 4096 pixels per core
PTILE = 512
NT = NPIX // PTILE           # 8 pixel tiles

BF16 = mybir.dt.bfloat16
F32 = mybir.dt.float32
NPBF16 = ml_dtypes.bfloat16

# fraction of relu/bias ops routed to ScalarE (rest go to VectorE)
ACT_SPLIT = {"h0": 2, "h1": 2}   # 1 of every ACT_SPLIT[..] ops on ScalarE


# ----------------------------------------------------------------------------
# host-side packing (layout only)
# ----------------------------------------------------------------------------

def _pack_weights(w0, b0, w1, b1, w2, b2):
    """Shared (core-independent) weight/bias packing. Returns dict of np arrays."""
    w0 = np.asarray(w0, np.float32)
    w1 = np.asarray(w1, np.float32)
    w2 = np.asarray(w2, np.float32)
    b0 = np.asarray(b0, np.float32)
    b1 = np.asarray(b1, np.float32)
    b2 = np.asarray(b2, np.float32)

    w0a = np.empty((128, 32, 128), np.float32)   # [k, group, m=fl*32+d]
    # chunk2 (K rows 128..146 + bias row) packed for 4-way row-tiled
    # concurrency: group g lives at partitions 32*(g%4)+k, cols g*128+m.
    # Row 32*(g%4)+19 carries b0 (the patch tile has ones there), so the
    # PSUM result already includes the bias and the relu op needs none.
    w0b = np.zeros((128, 32, 128), np.float32)
    for g in range(32):
        m = w0[:, :, 4 * g:4 * g + 4].transpose(0, 2, 1).reshape(K, 128)
        w0a[:, g, :] = m[:128]
        r = g % 4
        w0b[32 * r:32 * r + 19, g, :] = m[128:]
        w0b[32 * r + 19, g, :] = b0[:, 4 * g:4 * g + 4].T.reshape(128)

    w1bd = np.zeros((128, 32, 64), np.float32)   # [k=fl*32+d0, g, m=fl*16+d1]
    b1s = np.empty((128, 16), np.float32)
    for g in range(32):
        for fl in range(4):
            f = 4 * g + fl
            w1bd[fl * 32:(fl + 1) * 32, g, fl * 16:(fl + 1) * 16] = w1[:, :, f]
    for p in range(16):
        for half in range(2):
            g = 2 * p + half
            b1s[half * 64:(half + 1) * 64, p] = b1[:, 4 * g:4 * g + 4].T.reshape(64)

    w2bd = np.zeros((128, 16, 32), np.float32)   # [k=half*64+fl*16+d1, pair, col]
    for p in range(16):
        for half in range(2):
            for fl in range(4):
                f = 8 * p + half * 4 + fl
                col = f - 32 * (p // 4)
                w2bd[half * 64 + fl * 16:half * 64 + (fl + 1) * 16, p, col] = w2[:, 0, f]
    b2s = b2.reshape(128, 1).astype(np.float32)

    return {
        "w0a": w0a.reshape(128, 4096).astype(NPBF16),
        "w0b": w0b.reshape(128, 4096).astype(NPBF16),
        "w1bd": w1bd.reshape(128, 2048).astype(NPBF16),
        "w2bd": w2bd.reshape(128, 512).astype(NPBF16),
        "b1s": b1s, "b2s": b2s,
    }


def _im2col_T(x_core):
    """x_core (4,32,32,3) fp32 -> PT (147, 4096) with k=(kh*7+kw)*3+c."""
    xp = np.pad(np.asarray(x_core, np.float32), ((0, 0), (3, 3), (3, 3), (0, 0)))
    PT = np.empty((K, NPIX), np.float32)
    for kh in range(KH):
        for kw in range(KW):
            blk = xp[:, kh:kh + H, kw:kw + W, :]
            t = kh * 7 + kw
            PT[t * 3:t * 3 + 3] = blk.transpose(3, 0, 1, 2).reshape(3, NPIX)
    return PT


# ----------------------------------------------------------------------------
# device kernel
# ----------------------------------------------------------------------------

def _body(tc):
    nc = tc.nc
    Relu = mybir.ActivationFunctionType.Relu
    Add, Max = mybir.AluOpType.add, mybir.AluOpType.max

    pt1 = nc.dram_tensor("pt1", [128, NPIX], BF16, kind="ExternalInput").ap()
    pt2 = nc.dram_tensor("pt2", [128, NPIX], BF16, kind="ExternalInput").ap()
    w0a = nc.dram_tensor("w0a", [128, 4096], BF16, kind="ExternalInput").ap()
    w0b = nc.dram_tensor("w0b", [128, 4096], BF16, kind="ExternalInput").ap()
    w1bd = nc.dram_tensor("w1bd", [128, 2048], BF16, kind="ExternalInput").ap()
    w2bd = nc.dram_tensor("w2bd", [128, 512], BF16, kind="ExternalInput").ap()
    b1d = nc.dram_tensor("b1s", [128, 16], F32, kind="ExternalInput").ap()
    b2d = nc.dram_tensor("b2s", [128, 1], F32, kind="ExternalInput").ap()
    out = nc.dram_tensor("out", [128, NPIX], F32, kind="ExternalOutput").ap()

    with (
        tc.tile_pool(name="consts", bufs=1) as cpool,
        tc.tile_pool(name="h0", bufs=20) as h0pool,
        tc.tile_pool(name="h1", bufs=20) as h1pool,
        tc.tile_pool(name="outs", bufs=3) as opool,
        tc.tile_pool(name="l0p", bufs=3, space="PSUM") as l0pool,
        tc.tile_pool(name="l12p", bufs=2, space="PSUM") as l12pool,
    ):
        def load(ap, shape, dt, tag):
            t = cpool.tile(shape, dt, tag=tag)
            nc.sync.dma_start(t[:], ap)
            return t

        # Fine-grained staging: the first quad needs only w0a/w0b slice 0 and
        # patch tile 0, so load those first in small pieces — PE starts ~1us
        # in and the HAM clock ramps on real work instead of DMA waits.
        def load_slices(ap, n, width, dt, tag):
            tiles = []
            for i in range(n):
                t = cpool.tile([128, width], dt, tag=f"{tag}{i}")
                nc.sync.dma_start(t[:], ap[:, ts(i, width)])
                tiles.append(t)
            return tiles

        was = load_slices(w0a, 1, 1024, BF16, "w0a")    # groups 0-7
        wbs = load_slices(w0b, 1, 1024, BF16, "w0b")
        pt1s = load_slices(pt1, 1, PTILE, BF16, "pt1_")
        pt2s = load_slices(pt2, 1, PTILE, BF16, "pt2_")
        for i in range(1, 4):                            # groups 8-31
            wa_t = cpool.tile([128, 1024], BF16, tag=f"w0a{i}")
            nc.sync.dma_start(wa_t[:], w0a[:, ts(i, 1024)])
            was.append(wa_t)
            wb_t = cpool.tile([128, 1024], BF16, tag=f"w0b{i}")
            nc.sync.dma_start(wb_t[:], w0b[:, ts(i, 1024)])
            wbs.append(wb_t)
        w1s = load(w1bd, [128, 2048], BF16, "w1")
        w2s = load(w2bd, [128, 512], BF16, "w2")
        b1s = load(b1d, [128, 16], F32, "b1")
        b2s = load(b2d, [128, 1], F32, "b2")
        for t in range(1, NT):
            p = cpool.tile([128, PTILE], BF16, tag=f"pt1_{t}")
            nc.sync.dma_start(p[:], pt1[:, ts(t, PTILE)])
            pt1s.append(p)
            p2 = cpool.tile([128, PTILE], BF16, tag=f"pt2_{t}")
            nc.sync.dma_start(p2[:], pt2[:, ts(t, PTILE)])
            pt2s.append(p2)

        def relu(dst, src, bias, idx):
            # alternate whole tiles between ScalarE and VectorE
            if idx % 2 == 0:
                nc.scalar.activation(dst, src, Relu, bias=bias)
            else:
                nc.vector.tensor_scalar(dst, src, bias, 0.0, Add, Max)

        for t in range(NT):
            pix = ts(t, PTILE)
            # ---- layer 0: 8 quads of 4 filter-groups; two (128,1024) PSUM
            # tiles per quad (2 groups each, one per column half); chunk2
            # (K rows 128..146 + bias row) runs 4-way concurrent via
            # row-group tiling. Bias rides in the matmul, so one wide
            # bias-free relu op covers a whole tile.
            h0 = []       # 16 tiles (128,1024): groups (2j, 2j+1)
            for q in range(8):
                psA = l0pool.tile([128, 2 * PTILE], F32, tag="l0")
                psB = l0pool.tile([128, 2 * PTILE], F32, tag="l0")
                for r in range(4):
                    g = 4 * q + r
                    ps = psA if r < 2 else psB
                    dst = ps[:, ts(r % 2, PTILE)]
                    nc.tensor.matmul(dst, was[g // 8][:, ts(g % 8, 128)],
                                     pt1s[t][:], start=True, stop=False)
                for r in range(4):
                    g = 4 * q + r
                    ps = psA if r < 2 else psB
                    dst = ps[:, ts(r % 2, PTILE)]
                    nc.tensor.matmul(dst,
                                     wbs[g // 8][32 * r:32 * r + 20, ts(g % 8, 128)],
                                     pt2s[t][32 * r:32 * r + 20, :],
                                     start=False, stop=True,
                                     tile_position=(32 * r, 0))
                for j, ps in ((2 * q, psA), (2 * q + 1, psB)):
                    h = h0pool.tile([128, 2 * PTILE], BF16, tag="h0")
                    if j % 2 == 0:
                        nc.scalar.activation(h[:], ps[:], Relu)
                    else:
                        nc.vector.tensor_scalar_max(h[:], ps[:], 0.0)
                    h0.append(h)
            # ---- layer 1: 16 pairs of groups -> (128 = 8f*16, pix)
            h1 = []
            for p in range(16):
                ps = l12pool.tile([128, PTILE], F32, tag="l12")
                nc.tensor.matmul(ps[0:64, :], w1s[:, ts(2 * p, 64)],
                                 h0[p][:, 0:PTILE], start=True, stop=True)
                nc.tensor.matmul(ps[64:128, :], w1s[:, ts(2 * p + 1, 64)],
                                 h0[p][:, PTILE:], start=True, stop=True)
                h = h1pool.tile([128, PTILE], BF16, tag="h1")
                relu(h[:], ps[:], b1s[:, p:p + 1], p)
                h1.append(h)
            # ---- layer 2: 4 blocks of 32 filters; q-major order so the 4
            # blocks' matmuls hit disjoint PE column groups concurrently
            ps2 = l12pool.tile([128, PTILE], F32, tag="l12")
            for q in range(4):
                for jj in range(4):
                    p = 4 * jj + q
                    nc.tensor.matmul(ps2[32 * jj:32 * jj + 32, :],
                                     w2s[:, ts(p, 32)], h1[p][:],
                                     start=(q == 0), stop=(q == 3),
                                     tile_position=(0, 32 * jj))
            ot = opool.tile([128, PTILE], F32, tag="o")
            nc.scalar.activation(ot[:], ps2[:], Relu, bias=b2s[:, 0:1])
            nc.sync.dma_start(out[:, pix], ot[:])


_COMPILED = None


def _get_compiled():
    global _COMPILED
    if _COMPILED is None:
        import time as _time
        t0 = _time.time()
        nc = bacc.Bacc("TRN2", target_bir_lowering=False, debug=False,
                       num_devices=NCORES)
        with tile.TileContext(nc) as tc:
            _body(tc)
        t1 = _time.time()
        nc.compile()
        t2 = _time.time()
        print(f"[kernel] tile build+schedule {t1 - t0:.1f}s, bacc compile {t2 - t1:.1f}s",
              flush=True)
        _COMPILED = nc
    return _COMPILED


# ----------------------------------------------------------------------------
# public entry point
# ----------------------------------------------------------------------------

def kernel(x, w0, b0, w1, b1, w2, b2, _trace=False):
    x = np.asarray(x, np.float32)
    shared = _pack_weights(w0, b0, w1, b1, w2, b2)

    in_maps = []
    for k in range(NCORES):
        PT = _im2col_T(x[BPC * k:BPC * (k + 1)])
        m = dict(shared)
        m["pt1"] = PT[:128].astype(NPBF16)
        # chunk2 rows replicated at partitions 32r (4-way row tiling),
        # with a ones row at 32r+19 that carries b0 through the matmul
        pt2 = np.zeros((128, NPIX), np.float32)
        for r in range(4):
            pt2[32 * r:32 * r + 19] = PT[128:]
            pt2[32 * r + 19] = 1.0
        m["pt2"] = pt2.astype(NPBF16)
        in_maps.append(m)

    import time as _time
    nc = _get_compiled()
    t0 = _time.time()
    res = bass_utils.run_bass_kernel_spmd(
        nc, in_maps, core_ids=list(range(NCORES)), trace=_trace)
    print(f"[kernel] run_bass_kernel_spmd {_time.time() - t0:.1f}s", flush=True)

    outs = []
    for k in range(NCORES):
        oc = res.results[k]["out"]                     # (128, 4096) fp32
        outs.append(oc.reshape(F, BPC, H, W).transpose(1, 2, 3, 0))
    full = np.concatenate(outs, axis=0).astype(np.float32)
    if _trace:
        return full, res
    return full
